# revision 41
# baseline (speedup 1.0000x reference)
"""Causal self-attention on 8 Trainium2 NeuronCores.

Sharding (data + head parallel): core c handles batch b = c // 4 and the
4 heads [4g, 4g+4) where g = c % 4.  Each core projects q/k/v for its
heads (weights pre-sliced + pre-transposed on host), runs causal
attention, then the 4 cores of each batch AllGather the per-head
attention outputs (hd-major fp16) and each computes a disjoint
256-channel column slice of the output projection.

Pipeline notes:
- fp16 data path, fp32 PSUM accumulation.
- Attention runs in 512-column q-chunks; both heads of a pair share one
  score tile (bank-aligned halves) so each j-step needs a single exp.
- The softmax denominator is accumulated on PSUM partition 0 (the ones
  column of v sits at channel 0), so the normalization chain is
  reciprocal_approx_fast on [1, 1024] -> partition_broadcast ->
  one multiply.  No DMA hop, no full-precision reciprocal.
- PSUM budget (8 banks): score tile 2 banks x 2 bufs, two 2-bank
  attention accumulators (tags at/pp), with the pp accumulator shared
  by the q/k/v/output projections so they interleave with the
  ACT-bound attention loop and keep the tensor engine dense.
- AllGathers go per (pair, chunk): 8 small collectives of
  [128, 512] fp16 that overlap attention; each gathered 512-column
  block feeds an output-projection block as soon as it lands.

Layouts per core:
  xT    (1024, 2048)  x[b].T                       (d on partitions)
  wqkT  (1024, 512)   [ (Wq[rows]/8).T | Wk[rows].T ]
  wvT   (1024, 256)   Wv[rows].T
  woT   (1024, 256)   Wo[rows].T with rows permuted to the AllGather
                      order: [pair p=0: rank r: heads 4r,4r+1] then
                      [pair p=1: rank r: heads 4r+2,4r+3]
  mask  (128, 128)    upper-triangular ones (k <= q)
  outT  (256, 2048)   out[b][:, cols].T
"""

import numpy as np

B, S, D, H = 2, 2048, 1024, 16
HD = D // H              # 64
NCORES = 8
GROUP = 4                # cores per batch
LHEADS = 4               # heads per core
LCH = LHEADS * HD        # 256 local channels
KT = D // 128            # 8 contraction tiles
ST = S // 128            # 16 sequence tiles
PAIRS = 2                # head pairs per core
CHUNK = 512              # q columns per attention pass
NCH = S // CHUNK         # 4

_CACHE = {}


def _f16(a):
    return np.ascontiguousarray(a, dtype=np.float16)


def _build():
    import concourse.bacc as bacc
    import concourse.mybir as mybir
    import concourse.tile as tile

    f32 = mybir.dt.float32
    f16 = mybir.dt.float16
    Exp = mybir.ActivationFunctionType.Exp

    nc = bacc.Bacc(num_devices=NCORES)
    xT = nc.dram_tensor("xT", [D, S], f16, kind="ExternalInput")
    wqkT = nc.dram_tensor("wqkT", [D, 2 * LCH], f16, kind="ExternalInput")
    wvT = nc.dram_tensor("wvT", [D, LCH], f16, kind="ExternalInput")
    woT = nc.dram_tensor("woT", [D, LCH], f16, kind="ExternalInput")
    mask = nc.dram_tensor("mask", [128, 128], f16, kind="ExternalInput")
    outT = nc.dram_tensor("outT", [LCH, S], f16, kind="ExternalOutput")

    RG = [[0, 1, 2, 3], [4, 5, 6, 7]]

    with tile.TileContext(nc, num_cores=NCORES) as tc:
        with (
            tc.tile_pool(name="const", bufs=1) as const,
            tc.tile_pool(name="qkv", bufs=1) as qkv,
            tc.tile_pool(name="psum", bufs=1, space="PSUM") as psum,
            tc.tile_pool(name="dram", bufs=1, space="DRAM") as dram,
            tc.tile_pool(name="work", bufs=1) as work,
            tc.tile_pool(name="proj", bufs=1) as projp,
            tc.tile_pool(name="agp", bufs=1) as agp,
        ):
            mask_sb = const.tile([128, 128], f16)
            nc.sync.dma_start(mask_sb[:], mask[:])

            GATH = 2 * CHUNK     # columns per collective block
            NHB = S // GATH      # 2 gather blocks per pair
            # One gather per (pair, 1024-col block).  Sub-256KB collectives
            # measure 21-26us each — pure latency/barrier, flat in size —
            # so fewer, bigger gathers strictly win.
            cc_in = [[dram.tile([128, GATH], f16, name=f"ccin{p}{hb}")
                      for hb in range(NHB)] for p in range(PAIRS)]
            cc_out = [[dram.tile([GROUP * 128, GATH], f16, name=f"ccout{p}{hb}")
                       for hb in range(NHB)] for p in range(PAIRS)]

            qt = qkv.tile([128, PAIRS, S], f16)
            kt = qkv.tile([128, PAIRS, S], f16)
            v = qkv.tile([128, ST, LHEADS, 65], f16)
            nc.vector.memset(v[:, :, :, 0:1], 1.0)   # softmax-denominator ones

            # ---------------- input loads ----------------
            # The whole first phase (qk half-0 projections, v_proj j<8,
            # attention chunks 0-1) reads only x columns 0-1023, so load
            # those first: the startup DMA critical path drops from ~6MB
            # to ~3.5MB before the first score matmul.
            xt, wqk, wv = [], [], []
            for k in range(KT):
                tx = projp.tile([128, S], f16, name=f"xt{k}")
                nc.sync.dma_start(tx[:, 0:1024], xT[128 * k:128 * k + 128, 0:1024])
                xt.append(tx)
                tw = projp.tile([128, 2 * LCH], f16, name=f"wqk{k}")
                nc.sync.dma_start(tw[:], wqkT[128 * k:128 * k + 128, :])
                wqk.append(tw)
                tv = projp.tile([128, LCH], f16, name=f"wv{k}")
                nc.sync.dma_start(tv[:], wvT[128 * k:128 * k + 128, :])
                wv.append(tv)
            for k in range(KT):
                nc.sync.dma_start(
                    xt[k][:, 1024:2048], xT[128 * k:128 * k + 128, 1024:2048])
            wo = projp.tile([128, KT, LCH], f16)
            nc.sync.dma_start(wo[:], woT[:].rearrange("(k p) n -> p k n", p=128))

            def qk_proj(m, half):
                # m: 0,1 = q pair 0/1; 2,3 = k pair 0/1
                # pair-0 k-proj runs on "at" so the startup q/k projections
                # pace the input DMA in parallel instead of serializing on pp.
                dst = qt if m < 2 else kt
                pp = psum.tile([128, 1024], f32, tag=("at" if m == 2 else "pp"),
                               name=f"qk{m}{half}")
                for k in range(KT):
                    for c2 in range(2):
                        o = 1024 * half + 512 * c2
                        nc.tensor.matmul(
                            pp[:, 512 * c2:512 * c2 + 512],
                            wqk[k][:, 128 * m:128 * m + 128],
                            xt[k][:, o:o + 512],
                            start=(k == 0), stop=(k == KT - 1))
                nc.vector.tensor_copy(
                    dst[:, m % 2, 1024 * half:1024 * half + 1024], pp[:])

            def v_proj(j):
                vps = psum.tile([128, LCH], f32, tag=("at" if j % 2 == 0 else "pp"),
                                name=f"v{j}")
                for k in range(KT):
                    nc.tensor.matmul(
                        vps[:], xt[k][:, 128 * j:128 * j + 128], wv[k][:],
                        start=(k == 0), stop=(k == KT - 1))
                nc.vector.tensor_copy(
                    v[:, j, :, 1:65], vps[:].rearrange("p (h e) -> p h e", h=LHEADS))

            ag = {}
            stage_dma = {}
            last_trig = [None]

            def stage_chunk(p, c, attps):
                """Normalize chunk c (denominators sit on partition 0) and
                ship it to the collective buffer.  Pair-1 chunks stage per
                head-half so the two sub-chains pipeline across DVE/GpSimd
                and the gather trigger fires ~2us earlier."""
                hb, sub = c // 2, c % 2
                nh = 2 if p == 1 else 1
                w = 2 * CHUNK // nh
                for h in range(nh):
                    asb = work.tile([65, w], f32, tag=f"asb{h}", bufs=2,
                                    name=f"asb{p}{c}{h}")
                    nc.vector.tensor_copy(asb[:], attps[:, w * h:w * h + w])
                    rc = work.tile([1, w], f32, tag=f"rc{h}", bufs=2,
                                   name=f"rc{p}{c}{h}")
                    nc.vector.reciprocal_approx_fast(rc[0:1, :], asb[0:1, :])
                    bc = work.tile([65, w], f32, tag=f"bc{h}", bufs=2,
                                   name=f"bc{p}{c}{h}")
                    bcast = nc.gpsimd.partition_broadcast(bc[:], rc[0:1, :])
                    if last_trig[0] is not None and h == 0:
                        # Order-only edge: keep each AllGather trigger ahead
                        # of the next chunk's broadcast in the in-order
                        # GpSimd stream, so gathers fire as soon as staging
                        # lands instead of queueing behind later chains.
                        tile.add_dep_helper(bcast.ins, last_trig[0].ins,
                                            sync=False,
                                            reason="trigger before broadcast")
                    ao = work.tile([65, w], f16, tag=f"ao{h}", bufs=2,
                                   name=f"ao{p}{c}{h}")
                    # partition 0 computes d * 1/d (unused); DVE operands
                    # must start at an aligned partition -> full 65 rows.
                    nc.vector.tensor_mul(ao[0:65, :], asb[0:65, :], bc[0:65, :])
                    for hh in range(2 // nh):
                        hd_row = h if nh == 2 else hh
                        stage_dma[(p, c)] = nc.sync.dma_start(
                            cc_in[p][hb][64 * hd_row:64 * hd_row + 64,
                                         CHUNK * sub:CHUNK * sub + CHUNK],
                            ao[1:65, CHUNK * hh:CHUNK * hh + CHUNK])
                if sub == 1:
                    last_trig[0] = nc.gpsimd.collective_compute(
                        "AllGather", mybir.AluOpType.bypass, replica_groups=RG,
                        ins=[cc_in[p][hb][:]], outs=[cc_out[p][hb][:]])

            def attn_chunk(p, c):
                q0 = CHUNK * c
                nj = 4 * c + 4
                attps = psum.tile([65, 2 * CHUNK], f32,
                                  tag=("at" if c % 2 == 0 else "pp"),
                                  name=f"att{p}{c}")
                for j in range(nj):
                    qs = max(q0, 128 * j)
                    n = q0 + CHUNK - qs
                    off = qs - q0
                    sc = psum.tile([128, 1024], f32, tag="sc", bufs=2,
                                   name=f"sc{p}{c}{j}")
                    for h in range(2):
                        pb = 64 * h
                        nc.tensor.matmul(
                            sc[:, 512 * h:512 * h + n],
                            kt[pb:pb + 64, p, 128 * j:128 * j + 128],
                            qt[pb:pb + 64, p, qs:qs + n],
                            start=True, stop=True)
                    ex = work.tile([128, 1024], f16, tag="ex", bufs=3,
                                   name=f"ex{p}{c}{j}")
                    # One flat activation over both head halves: a 2D AP keeps
                    # this a single ACT instruction (the sliced 3D AP splits
                    # into two, costing ~300ns fixed overhead each).  Columns
                    # beyond n hold exp(stale PSUM) and are never read.
                    nc.scalar.activation(ex[:, :], sc[:, :], Exp)
                    if qs == 128 * j:  # diagonal tile: causal mask
                        for h in range(2):
                            nc.vector.tensor_mul(
                                ex[:, 512 * h:512 * h + 128],
                                ex[:, 512 * h:512 * h + 128], mask_sb[:])
                    for h in range(2):
                        nc.tensor.matmul(
                            attps[:, 512 * h + off:512 * h + CHUNK],
                            v[:, j, 2 * p + h, :],
                            ex[:, 512 * h:512 * h + n],
                            start=(j == 0), stop=(j == nj - 1))
                stage_chunk(p, c, attps)

            def out_block(hb):
                # One 1024-column output-projection block per gather block.
                # The scheduler's cost model doesn't know collective latency;
                # pin each prefetch after the staging DMA of the last chunk
                # feeding its own gather, so a slow AllGather can't stall the
                # in-order SP queue ahead of later staging DMAs.  Pair-0
                # prefetches unpin early, letting the k=0..3 half of the
                # accumulation run while pair-1 gathers are still in flight.
                for p in range(PAIRS):
                    for r in range(GROUP):
                        t = agp.tile([128, GATH], f16, name=f"ag{p}{hb}{r}")
                        dma = nc.sync.dma_start(
                            t[:], cc_out[p][hb][128 * r:128 * r + 128, :])
                        tile.add_dep_helper(
                            dma.ins, stage_dma[(p, 2 * hb + 1)].ins, sync=True,
                            reason="gather prefetch after staging")
                        ag[(p, hb, r)] = t
                # hb=0 uses "at" (freed mid-tail-attention); hb=1 uses "sc"
                # (freed right after the last exp) so neither waits on the
                # pp-held final attention accumulator.
                def mm(pp, ct, k, c2):
                    nc.tensor.matmul(
                        pp[:, 512 * c2:512 * c2 + 512],
                        wo[:, k, 128 * ct:128 * ct + 128],
                        ag[(k // 4, hb, k % 4)][:, 512 * c2:512 * c2 + 512],
                        start=(k == 0), stop=(k == KT - 1))

                def out_store(pp, ct):
                    ot = agp.tile([128, GATH], f16, tag=f"ot{ct}", bufs=2,
                                  name=f"ot{hb}{ct}")
                    nc.vector.tensor_copy(ot[:], pp[:])
                    nc.sync.dma_start(
                        outT[128 * ct:128 * ct + 128,
                             GATH * hb:GATH * hb + GATH], ot[:])

                if hb == 0:
                    for ct in range(2):
                        pp = psum.tile([128, GATH], f32, tag="at",
                                       name=f"op{hb}{ct}")
                        for k in range(KT):
                            for c2 in range(2):
                                mm(pp, ct, k, c2)
                        out_store(pp, ct)
                else:
                    # Final block: run both ct accumulations k-outer so all
                    # 16 pair-0 matmuls (no dependency on the final gather)
                    # execute while the last AllGather is in flight.
                    pps = [psum.tile([128, GATH], f32, tag="sc", bufs=2,
                                     name=f"op{hb}{ct}") for ct in range(2)]
                    for k in range(4):
                        for ct in range(2):
                            for c2 in range(2):
                                mm(pps[ct], ct, k, c2)
                    for k in range(4, KT):
                        for ct in range(2):
                            for c2 in range(2):
                                mm(pps[ct], ct, k, c2)
                    for ct in range(2):
                        out_store(pps[ct], ct)

            # ---------------- schedule ----------------
            qk_proj(0, 0)         # pair-0 q, cols 0-1023
            qk_proj(2, 0)         # pair-0 k, cols 0-1023
            for j in range(4):
                v_proj(j)
            attn_chunk(0, 0)
            for j in range(4, 8):
                v_proj(j)
            attn_chunk(0, 1)
            qk_proj(0, 1)         # pair-0 q, cols 1024-2047
            qk_proj(2, 1)         # pair-0 k, cols 1024-2047
            for j in range(8, ST):
                v_proj(j)
            qk_proj(1, 0)         # pair-1 q
            qk_proj(1, 1)
            attn_chunk(0, 2)
            qk_proj(3, 0)         # pair-1 k
            qk_proj(3, 1)
            attn_chunk(0, 3)
            for c in range(NCH):
                attn_chunk(1, c)
            for hb in range(NHB):
                out_block(hb)

    nc.compile()
    return nc


def _gather_perm():
    """d-channel permutation matching the AllGather layout."""
    perm = []
    for p in range(PAIRS):
        for r in range(GROUP):
            for h in range(2):
                head = 4 * r + 2 * p + h
                perm.extend(range(HD * head, HD * head + HD))
    return np.array(perm)


def _shard_inputs(x, Wq, Wk, Wv, Wo):
    x = np.asarray(x, dtype=np.float32)
    Wq = np.asarray(Wq, dtype=np.float32)
    Wk = np.asarray(Wk, dtype=np.float32)
    Wv = np.asarray(Wv, dtype=np.float32)
    Wo = np.asarray(Wo, dtype=np.float32)
    mask = np.triu(np.ones((128, 128), dtype=np.float16))
    perm = _gather_perm()
    in_maps = []
    for c in range(NCORES):
        b, g = c // GROUP, c % GROUP
        rows = slice(LCH * g, LCH * g + LCH)
        in_maps.append({
            "xT": _f16(x[b].T),
            "wqkT": _f16(np.concatenate([Wq[rows] / 8.0, Wk[rows]], axis=0).T),
            "wvT": _f16(Wv[rows].T),
            "woT": _f16(Wo[rows].T[perm, :]),
            "mask": mask,
        })
    return in_maps


def kernel(x, Wq, Wk, Wv, Wo):
    from concourse.bass_utils import run_bass_kernel_spmd

    if "nc" not in _CACHE:
        _CACHE["nc"] = _build()
    nc = _CACHE["nc"]
    in_maps = _shard_inputs(x, Wq, Wk, Wv, Wo)
    res = run_bass_kernel_spmd(nc, in_maps, core_ids=list(range(NCORES)))
    _CACHE["last_results"] = res
    out = np.empty((B, S, D), dtype=np.float32)
    for c in range(NCORES):
        b, g = c // GROUP, c % GROUP
        out[b][:, LCH * g:LCH * g + LCH] = res.results[c]["outT"].T.astype(np.float32)
    return out
